# revision 10
# baseline (speedup 1.0000x reference)
"""Per-image 256-bin luma-histogram entropy on Trainium2 (Bass, 8-core SPMD).

Input  x: (32, 3, 512, 512) fp32 RGB in [0,1]
Output   : (32,) fp32 entropy scores

Sharding: pure data parallel — batch split 4 images per NeuronCore, no
cross-core communication.

Per-core algorithm (4 images, processed as 8 half-images of [128,1024]):
  y = (0.299 R + 0.587 G + 0.114 B) * 255, RNE-round -> u in [0,255] (int16)
  hi = u & 0xF0, lo = u & 0x0F
  one-hot planes (bf16): Hi_j = (hi == 16j), Lo_j = (lo == j), j = 0..15
  histogram on TensorE: for each 8-column pixel group g,
      lhsT[k, (c,j)] = Hi_j[k, col 8g+c]   [K=128, M=8x16]
      rhs [k, (c,j)] = Lo_j[k, col 8g+c]   [K=128, N=8x16]
    accumulate into PSUM[128,128]; diagonal 16x16 blocks (c==c') hold the
    true joint (hi,lo) counts, off-diagonal blocks are garbage.
  fold: mask off-diagonal blocks (DVE multiply by block-diag 0/1 mask),
    selector matmul (tiled I16) sums blocks vertically, grouped free-dim
    reduce sums horizontally -> hist[16,16] counts per image.
  entropy: ACT Ln(c/N + eps); e = c * ln; reduce; ones-matmul partition fold;
    score = -sum(e) / (N*ln2).
"""

from contextlib import ExitStack

import numpy as np

N_IMG = 4  # images per core
N_CORES = 8
H = 512
W = 512
P = 128  # SBUF partitions
HALF = 1024  # pixel columns per half-image ([128, 1024] = 131072 px)
NPIX = H * W  # pixels per image
EPS = 1e-8
LN2 = 0.6931471805599453
MAGIC = 12582912.0  # 1.5 * 2**23: add+sub performs fp32 round-to-nearest-even
NHALF = N_IMG * 2
NGRP = HALF // 8  # 8-column matmul groups per half (128 cols each op)


def build_bass(reps=1):
    """Build the per-core Bass program. reps>1 repeats the whole pipeline
    (for marginal-cost HW timing); semaphore thresholds are offset per rep."""
    import concourse.bass as bass
    import concourse.mybir as mybir

    f32 = mybir.dt.float32
    bf16 = mybir.dt.bfloat16
    i16 = mybir.dt.int16
    Alu = mybir.AluOpType
    Act = mybir.ActivationFunctionType
    Axis = mybir.AxisListType

    nc = bass.Bass()

    x_t = nc.dram_tensor("x", [N_IMG, 3, H, W], f32, kind="ExternalInput")
    sel_t = nc.dram_tensor("sel", [P, 16], f32, kind="ExternalInput")
    mask_t = nc.dram_tensor("mask", [P, P], f32, kind="ExternalInput")
    ones_t = nc.dram_tensor("ones16", [16, 1], f32, kind="ExternalInput")
    out_t = nc.dram_tensor("out", [N_IMG], f32, kind="ExternalOutput")

    ctx = ExitStack()
    with ctx:
        # SBUF
        rgb = [
            ctx.enter_context(nc.sbuf_tensor(f"rgb{n}", [P, 3 * HALF], f32))
            for n in range(2)
        ]
        t_a = ctx.enter_context(nc.sbuf_tensor("t_a", [P, HALF], f32))
        t_b = ctx.enter_context(nc.sbuf_tensor("t_b", [P, HALF], f32))
        u16 = ctx.enter_context(nc.sbuf_tensor("u16", [P, HALF], i16))
        vhi = ctx.enter_context(nc.sbuf_tensor("vhi", [P, HALF], i16))
        vlo = ctx.enter_context(nc.sbuf_tensor("vlo", [P, HALF], i16))
        hi_b = [
            ctx.enter_context(nc.sbuf_tensor(f"hi{n}", [P, 16 * HALF], bf16))
            for n in range(2)
        ]
        lo_b = [
            ctx.enter_context(nc.sbuf_tensor(f"lo{n}", [P, 16 * HALF], bf16))
            for n in range(2)
        ]
        sel_sb = ctx.enter_context(nc.sbuf_tensor("sel_sb", [P, 16], f32))
        mask_sb = ctx.enter_context(nc.sbuf_tensor("mask_sb", [P, P], f32))
        ones_sb = ctx.enter_context(nc.sbuf_tensor("ones_sb", [16, 1], f32))
        p_sb = [
            ctx.enter_context(nc.sbuf_tensor(f"p_sb{n}", [P, P], f32))
            for n in range(2)
        ]
        hist4 = ctx.enter_context(nc.sbuf_tensor("hist4", [16, 16 * N_IMG], f32))
        ln4 = ctx.enter_context(nc.sbuf_tensor("ln4", [16, 16 * N_IMG], f32))
        e4 = ctx.enter_context(nc.sbuf_tensor("e4", [16, 16 * N_IMG], f32))
        part = ctx.enter_context(nc.sbuf_tensor("part", [16, N_IMG], f32))
        score_sb = ctx.enter_context(nc.sbuf_tensor("score_sb", [N_IMG, 1], f32))
        warm = ctx.enter_context(nc.sbuf_tensor("warm", [1, 2], f32))
        eps_sb = ctx.enter_context(nc.sbuf_tensor("eps_sb", [16, 1], f32))

        # PSUM
        psum_h = [
            ctx.enter_context(nc.psum_tensor(f"psum_h{n}", [P, P], f32))
            for n in range(N_IMG)
        ]
        psum_o = [
            ctx.enter_context(nc.psum_tensor(f"psum_o{n}", [16, P], f32))
            for n in range(2)
        ]
        psum_s = ctx.enter_context(nc.psum_tensor("psum_s", [N_IMG, 1], f32))

        # semaphores
        sem_dma = [
            ctx.enter_context(nc.semaphore(f"dma_in{n}")) for n in range(2)
        ]
        sem_cdma = ctx.enter_context(nc.semaphore("const_dma"))
        sem_rgbf = ctx.enter_context(nc.semaphore("rgb_free"))
        sem_pl = ctx.enter_context(nc.semaphore("planes"))
        sem_peh = ctx.enter_context(nc.semaphore("pe_half"))
        sem_psb = ctx.enter_context(nc.semaphore("psb"))
        sem_smm = ctx.enter_context(nc.semaphore("selmm"))
        sem_red = ctx.enter_context(nc.semaphore("red"))
        sem_ln = ctx.enter_context(nc.semaphore("ln"))
        sem_part = ctx.enter_context(nc.semaphore("part"))
        sem_sm = ctx.enter_context(nc.semaphore("scoremm"))
        sem_sc = ctx.enter_context(nc.semaphore("score"))
        sem_out = ctx.enter_context(nc.semaphore("out_dma"))
        sem_v = ctx.enter_context(nc.semaphore("dve_chain"))
        sem_wm = ctx.enter_context(nc.semaphore("warm"))

        def x_half_ap(i, c, h):
            # [512,512] -> [128, 2048] (4 consecutive rows per partition), half h
            a = x_t[i, c].rearrange("(p r) w -> p (r w)", r=4)
            return a[:, h * HALF : (h + 1) * HALF]

        with nc.Block() as block:

            @block.sync
            def _(sync):
                sync.dma_start(out=sel_sb[:], in_=sel_t[:]).then_inc(sem_cdma, 16)
                sync.dma_start(out=mask_sb[:], in_=mask_t[:]).then_inc(sem_cdma, 16)
                sync.dma_start(out=ones_sb[:], in_=ones_t[:]).then_inc(sem_cdma, 16)
                for r in range(reps):
                    for k in range(NHALF):
                        i, h = divmod(k, 2)
                        b = k % 2
                        gh = r * NHALF + k
                        if gh >= 2:
                            sync.wait_ge(sem_rgbf, gh - 1)
                        for c in range(3):
                            sync.dma_start(
                                out=rgb[b][:, c * HALF : (c + 1) * HALF],
                                in_=x_half_ap(i, c, h),
                            ).then_inc(sem_dma[b], 16)
                sync.wait_ge(sem_sc, reps)
                sync.dma_start(out=out_t[:], in_=score_sb[:, 0:1]).then_inc(
                    sem_out, 16
                )
                sync.wait_ge(sem_out, 16)

            @block.vector
            def _(vector):
                # Same-engine RAW/WAR requires explicit sem edges (engine
                # write-completion is async w.r.t. next-instruction issue;
                # cross-engine consumers inherit per-engine completion order).
                # Each DVE op incs exactly one sem: sem_v by default, or its
                # cross-engine signal sem. Same-engine waits reference
                # whichever sem the producer inc'd.
                vcnt = 0

                def vop(inst, sem=None, val=1):
                    nonlocal vcnt
                    if sem is None:
                        inst.then_inc(sem_v, 1)
                        vcnt += 1
                    else:
                        inst.then_inc(sem, val)
                    return inst

                def vwait():
                    vector.wait_ge(sem_v, vcnt)

                vop(vector.memset(warm[:], 1.0), sem=sem_wm)
                vop(vector.memset(eps_sb[:], EPS))
                r = 0
                for k in range(reps * NHALF):
                    r, k = divmod(k, NHALF)
                    gh = r * NHALF + k
                    b = k % 2
                    vector.wait_ge(sem_dma[b], 48 * (gh // 2 + 1))
                    if gh >= 2:
                        vector.wait_ge(sem_peh, gh - 1)  # plane bufs b free
                    vwait()  # prior half's DVE work (WAR on t_a/t_b/u16/...)
                    R = rgb[b][:, 0:HALF]
                    G = rgb[b][:, HALF : 2 * HALF]
                    B = rgb[b][:, 2 * HALF : 3 * HALF]
                    vop(vector.tensor_scalar_mul(t_a[:], R, 0.299))
                    vwait()
                    vop(
                        vector.scalar_tensor_tensor(
                            t_b[:], G, 0.587, t_a[:], Alu.mult, Alu.add
                        )
                    )
                    vwait()
                    vop(
                        vector.scalar_tensor_tensor(
                            t_a[:], B, 0.114, t_b[:], Alu.mult, Alu.add
                        ),
                        sem=sem_rgbf,
                    )
                    vector.wait_ge(sem_rgbf, gh + 1)
                    vop(
                        vector.tensor_scalar(
                            t_b[:], t_a[:], 255.0, MAGIC, Alu.mult, Alu.add
                        )
                    )
                    vwait()
                    vop(
                        vector.tensor_scalar(
                            u16[:], t_b[:], MAGIC, None, Alu.subtract
                        )
                    )
                    vwait()
                    vop(
                        vector.tensor_scalar(vhi[:], u16[:], 240, None, Alu.bitwise_and)
                    )
                    vop(
                        vector.tensor_scalar(vlo[:], u16[:], 15, None, Alu.bitwise_and)
                    )
                    vwait()  # vhi/vlo ready for all plane ops
                    # blocked-interleaved plane layout: free index = g*128+j*8+c
                    # (g = 8-col group, j = plane, c = col within group) so each
                    # matmul operand is one contiguous 128-elem slice and the
                    # DVE write keeps an innermost step-1 run of 8.
                    hi_w = hi_b[b][:].rearrange("p (g j c) -> p g j c", j=16, c=8)
                    lo_w = lo_b[b][:].rearrange("p (g j c) -> p g j c", j=16, c=8)
                    for j in range(16):
                        vop(
                            vector.tensor_scalar(
                                hi_w[:, :, j, :],
                                vhi[:],
                                16 * j,
                                None,
                                Alu.is_equal,
                            )
                        )
                        inst = vector.tensor_scalar(
                            lo_w[:, :, j, :],
                            vlo[:],
                            j,
                            None,
                            Alu.is_equal,
                        )
                        if j == 15:
                            vop(inst, sem=sem_pl)  # half done -> PE may start
                        else:
                            vop(inst)

                    # ---- tail (after last half of this rep) ----
                    if k != NHALF - 1:
                        continue
                    vector.wait_ge(sem_peh, (r + 1) * NHALF)  # all hist MMs done
                    vector.wait_ge(sem_cdma, 48)  # mask loaded
                    for i in range(N_IMG):
                        gi = r * N_IMG + i
                        if gi >= 2:
                            vector.wait_ge(sem_smm, gi - 1)  # p_sb[i%2] free
                        vop(
                            vector.tensor_tensor(
                                p_sb[i % 2][:], psum_h[i][:], mask_sb[:], Alu.mult
                            ),
                            sem=sem_psb,
                        )
                    if r >= 1:
                        vector.wait_ge(sem_ln, r)  # prior rep's ACT read of hist4
                    for i in range(N_IMG):
                        gi = r * N_IMG + i
                        vector.wait_ge(sem_smm, gi + 1)
                        src = psum_o[i % 2][:].rearrange("j (l c) -> j l c", c=8)
                        vop(
                            vector.tensor_reduce(
                                hist4[:, 16 * i : 16 * (i + 1)], src, Axis.X, Alu.add
                            ),
                            sem=sem_red,
                        )
                    vector.wait_ge(sem_red, (r + 1) * N_IMG)  # reduces done (hist4)
                    vector.wait_ge(sem_ln, r + 1)
                    vop(vector.tensor_tensor(e4[:], hist4[:], ln4[:], Alu.mult))
                    vwait()
                    vop(
                        vector.tensor_reduce(
                            part[:],
                            e4[:].rearrange("p (i l) -> p i l", i=N_IMG),
                            Axis.X,
                            Alu.add,
                        ),
                        sem=sem_part,
                    )
                    vector.wait_ge(sem_sm, r + 1)
                    vop(
                        vector.tensor_scalar(
                            score_sb[:], psum_s[:], -1.0 / (NPIX * LN2), None, Alu.mult
                        ),
                        sem=sem_sc,
                    )

            @block.tensor
            def _(tensor):
                for r in range(reps):
                    for k in range(NHALF):
                        i, h = divmod(k, 2)
                        b = k % 2
                        gh = r * NHALF + k
                        tensor.wait_ge(sem_pl, gh + 1)
                        if h == 0 and r >= 1:
                            # psum_h[i] free only after prior rep's mask-mult
                            tensor.wait_ge(sem_psb, (r - 1) * N_IMG + i + 1)
                        last = None
                        for g in range(NGRP):
                            last = tensor.matmul(
                                psum_h[i][:],
                                lhsT=hi_b[b][:, 128 * g : 128 * (g + 1)],
                                rhs=lo_b[b][:, 128 * g : 128 * (g + 1)],
                                start=(h == 0 and g == 0),
                                stop=(h == 1 and g == NGRP - 1),
                            )
                        last.then_inc(sem_peh, 1)

                    tensor.wait_ge(sem_cdma, 48)
                    for i in range(N_IMG):
                        gi = r * N_IMG + i
                        tensor.wait_ge(sem_psb, gi + 1)
                        if gi >= 2:
                            tensor.wait_ge(sem_red, gi - 1)  # psum_o[i%2] free
                        tensor.matmul(
                            psum_o[i % 2][:],
                            lhsT=sel_sb[:],
                            rhs=p_sb[i % 2][:],
                            start=True,
                            stop=True,
                        ).then_inc(sem_smm, 1)
                    tensor.wait_ge(sem_part, r + 1)
                    if r >= 1:
                        tensor.wait_ge(sem_sc, r)  # psum_s free after DVE read
                    tensor.matmul(
                        psum_s[:], lhsT=part[:], rhs=ones_sb[:], start=True, stop=True
                    ).then_inc(sem_sm, 1)

            @block.scalar
            def _(scalar):
                # warm up the Ln table early (overlaps with main pipeline)
                scalar.wait_ge(sem_wm, 1)
                scalar.activation(warm[:], warm[:], Act.Ln, bias=1.0, scale=0.0)
                for r in range(reps):
                    scalar.wait_ge(sem_red, (r + 1) * N_IMG)
                    scalar.activation(
                        ln4[:], hist4[:], Act.Ln, bias=eps_sb[:], scale=1.0 / NPIX
                    ).then_inc(sem_ln, 1)

    return nc


_NC_CACHE = {}


def _get_nc(reps=1):
    if reps not in _NC_CACHE:
        _NC_CACHE[reps] = build_bass(reps)
    return _NC_CACHE[reps]


def consts():
    # blocked layout: psum index m = j*8 + c (j = plane, c = col-in-group)
    sel = np.zeros((P, 16), np.float32)
    for k in range(P):
        sel[k, k // 8] = 1.0
    mask = np.zeros((P, P), np.float32)
    for k in range(P):
        mask[k, k % 8 :: 8] = 1.0
    ones16 = np.ones((16, 1), np.float32)
    return sel, mask, ones16


def kernel(x):
    x = np.ascontiguousarray(np.asarray(x, dtype=np.float32))
    assert x.shape == (N_IMG * N_CORES, 3, H, W)
    from concourse.bass_utils import run_bass_kernel_spmd

    nc = _get_nc()
    sel, mask, ones16 = consts()
    in_maps = [
        {
            "x": np.ascontiguousarray(x[N_IMG * i : N_IMG * (i + 1)]),
            "sel": sel,
            "mask": mask,
            "ones16": ones16,
        }
        for i in range(N_CORES)
    ]
    res = run_bass_kernel_spmd(nc, in_maps, core_ids=list(range(N_CORES)))
    return np.concatenate([res.results[i]["out"] for i in range(N_CORES)])


# revision 18
# speedup vs baseline: 25.1463x; 25.1463x over previous
"""Per-image 256-bin luma-histogram entropy on Trainium2 (Bass, 8-core SPMD).

Input  x: (32, 3, 512, 512) fp32 RGB in [0,1]
Output   : (32,) fp32 entropy scores

Sharding: pure data parallel — batch split 4 images per NeuronCore, no
cross-core communication.

Per-core algorithm (4 images, processed as 8 half-images of [128,1024]):
  y = (0.299 R + 0.587 G + 0.114 B), m = y*255 RNE-rounded -> u in [0,255]
  (int16, RNE via the +-1.5*2^23 magic-add trick).

  Histogram via step-function factor planes (bf16, blocked layout
  free index = g*128 + t*8 + c, g = 8-col group, t = plane, c = col):
    hi side (planes t=0..15):  f_0 = 1 (one-time memset);
       t in DVE_HI:  f_t = (u >= 16t)          [DVE is_ge, {0,1}]
       t in ACT_HI:  f_t = sign(255*y-(16t-.5)) [ScalarE Sign, {-1,+1}]
    lo side (planes s=0..15):  g_0 = 1 (memset);
       g_s = ((u & 15) >= s)                   [DVE fused and+is_ge]
  TensorE contracts 8-col groups: lhsT/rhs = contiguous 128-col slices of
  the hi/lo plane buffers, accumulating PSUM[128,128]; entries with c==c'
  hold M_c[t,s] partial sums, c!=c' blocks are garbage.
  Fold: DVE multiply by block-diag mask (c==c'), then selector matmul
  whose constant bakes in W = F^-1 (recovery of hi one-hot counts from the
  mixed step/sign family), then grouped free-dim reduce over c' -> M'[j,s].
  Lo recovery is a column difference: J[:,s] = M'[:,s] - M'[:,s+1],
  J[:,15] = M'[:,15]  (G is the step family).
  entropy: ACT Ln(J/N + eps); e = J * ln; reduce; ones-matmul partition
  fold; score = -sum(e) / (N*ln2).

Engine sync: same-engine RAW/WAR needs explicit sem edges (engine
write-completion is async w.r.t. next-instruction issue; cross-engine
consumers inherit per-engine completion order). Each DVE op incs exactly
one sem: sem_v by default, or its cross-engine signal sem.
"""

from contextlib import ExitStack

import numpy as np

N_IMG = 4  # images per core
N_CORES = 8
H = 512
W = 512
P = 128  # SBUF partitions
HALF = 1024  # pixel columns per half-image ([128, 1024] = 131072 px)
NPIX = H * W  # pixels per image
EPS = 1e-8
LN2 = 0.6931471805599453
MAGIC = 12582912.0  # 1.5 * 2**23: add+sub performs fp32 round-to-nearest-even
CR = float(np.float32(0.299) / np.float32(0.587))
CB = float(np.float32(0.114) / np.float32(0.587))
YSCL = float(np.float32(0.587) * np.float32(255.0))
NHALF = N_IMG * 2
NGRP = HALF // 8  # 8-column matmul groups per half (128 cols each op)

# hi-plane split between engines (t=1..15; t=0 is the memset ones plane)
ACT_HI = tuple(range(7, 16))  # planes computed on ScalarE as sign (+-1)
DVE_HI = tuple(t for t in range(1, 16) if t not in ACT_HI)


def build_bass(reps=1):
    """Build the per-core Bass program. reps>1 repeats the whole pipeline
    (for marginal-cost HW timing); semaphore thresholds are offset per rep."""
    import concourse.bass as bass
    import concourse.mybir as mybir

    f32 = mybir.dt.float32
    bf16 = mybir.dt.bfloat16
    i16 = mybir.dt.int16
    Alu = mybir.AluOpType
    Act = mybir.ActivationFunctionType
    Axis = mybir.AxisListType

    nc = bass.Bass()

    x_t = nc.dram_tensor("x", [N_IMG, 3, H, W], f32, kind="ExternalInput")
    sel_t = nc.dram_tensor("sel", [P, 16], f32, kind="ExternalInput")
    mask_t = nc.dram_tensor("mask", [P, P], f32, kind="ExternalInput")
    ones_t = nc.dram_tensor("ones16", [16, 1], f32, kind="ExternalInput")
    out_t = nc.dram_tensor("out", [N_IMG], f32, kind="ExternalOutput")

    ctx = ExitStack()
    with ctx:
        # SBUF
        rgb = [
            ctx.enter_context(nc.sbuf_tensor(f"rgb{n}", [P, 3 * HALF], f32))
            for n in range(2)
        ]
        t_a = ctx.enter_context(nc.sbuf_tensor("t_a", [P, HALF], f32))
        t_y = [
            ctx.enter_context(nc.sbuf_tensor(f"t_y{n}", [P, HALF], f32))
            for n in range(2)
        ]
        u16 = ctx.enter_context(nc.sbuf_tensor("u16", [P, HALF], i16))
        vlo = ctx.enter_context(nc.sbuf_tensor("vlo", [P, HALF], i16))
        hi_b = [
            ctx.enter_context(nc.sbuf_tensor(f"hi{n}", [P, 16 * HALF], bf16))
            for n in range(2)
        ]
        lo_b = [
            ctx.enter_context(nc.sbuf_tensor(f"lo{n}", [P, 16 * HALF], bf16))
            for n in range(2)
        ]
        sel_sb = ctx.enter_context(nc.sbuf_tensor("sel_sb", [P, 16], f32))
        mask_sb = ctx.enter_context(nc.sbuf_tensor("mask_sb", [P, P], f32))
        ones_sb = ctx.enter_context(nc.sbuf_tensor("ones_sb", [16, 1], f32))
        p_sb = [
            ctx.enter_context(nc.sbuf_tensor(f"p_sb{n}", [P, P], f32))
            for n in range(2)
        ]
        mm4 = ctx.enter_context(nc.sbuf_tensor("mm4", [16, 16], f32))
        hist4 = ctx.enter_context(nc.sbuf_tensor("hist4", [16, 16 * N_IMG], f32))
        ln4 = ctx.enter_context(nc.sbuf_tensor("ln4", [16, 16 * N_IMG], f32))
        e4 = ctx.enter_context(nc.sbuf_tensor("e4", [16, 16 * N_IMG], f32))
        part = ctx.enter_context(nc.sbuf_tensor("part", [16, N_IMG], f32))
        score_sb = ctx.enter_context(nc.sbuf_tensor("score_sb", [N_IMG, 1], f32))
        warm = ctx.enter_context(nc.sbuf_tensor("warm", [1, 2], f32))
        eps_sb = ctx.enter_context(nc.sbuf_tensor("eps_sb", [16, 1], f32))
        bias_sb = ctx.enter_context(
            nc.sbuf_tensor("bias_sb", [P, len(ACT_HI)], f32)
        )

        # PSUM
        psum_h = [
            ctx.enter_context(nc.psum_tensor(f"psum_h{n}", [P, P], f32))
            for n in range(N_IMG)
        ]
        psum_o = [
            ctx.enter_context(nc.psum_tensor(f"psum_o{n}", [16, P], f32))
            for n in range(2)
        ]
        psum_s = ctx.enter_context(nc.psum_tensor("psum_s", [N_IMG, 1], f32))

        # semaphores
        sem_dma = [
            ctx.enter_context(nc.semaphore(f"dma_in{n}")) for n in range(2)
        ]
        sem_cdma = ctx.enter_context(nc.semaphore("const_dma"))
        sem_rgbf = ctx.enter_context(nc.semaphore("rgb_free"))
        sem_pl = ctx.enter_context(nc.semaphore("planes"))
        sem_pla = ctx.enter_context(nc.semaphore("planes_act"))
        sem_peh = ctx.enter_context(nc.semaphore("pe_half"))
        sem_psb = ctx.enter_context(nc.semaphore("psb"))
        sem_smm = ctx.enter_context(nc.semaphore("selmm"))
        sem_red = ctx.enter_context(nc.semaphore("red"))
        sem_ln = ctx.enter_context(nc.semaphore("ln"))
        sem_part = ctx.enter_context(nc.semaphore("part"))
        sem_sm = ctx.enter_context(nc.semaphore("scoremm"))
        sem_sc = ctx.enter_context(nc.semaphore("score"))
        sem_out = ctx.enter_context(nc.semaphore("out_dma"))
        sem_v = ctx.enter_context(nc.semaphore("dve_chain"))
        sem_wm = ctx.enter_context(nc.semaphore("warm"))

        def x_half_ap(i, c, h):
            # [512,512] -> [128, 2048] (4 consecutive rows per partition), half h
            a = x_t[i, c].rearrange("(p r) w -> p (r w)", r=4)
            return a[:, h * HALF : (h + 1) * HALF]

        def plane(buf, t):
            # blocked plane slot t of a hi/lo buffer: [128, NGRP, 8] strided
            return buf[:].rearrange("p (g j c) -> p g j c", j=16, c=8)[:, :, t, :]

        with nc.Block() as block:

            @block.sync
            def _(sync):
                sync.dma_start(out=sel_sb[:], in_=sel_t[:]).then_inc(sem_cdma, 16)
                sync.dma_start(out=mask_sb[:], in_=mask_t[:]).then_inc(sem_cdma, 16)
                sync.dma_start(out=ones_sb[:], in_=ones_t[:]).then_inc(sem_cdma, 16)
                for r in range(reps):
                    for k in range(NHALF):
                        i, h = divmod(k, 2)
                        b = k % 2
                        gh = r * NHALF + k
                        if gh >= 2:
                            sync.wait_ge(sem_rgbf, gh - 1)
                        for c in range(3):
                            sync.dma_start(
                                out=rgb[b][:, c * HALF : (c + 1) * HALF],
                                in_=x_half_ap(i, c, h),
                            ).then_inc(sem_dma[b], 16)
                sync.wait_ge(sem_sc, reps)
                sync.dma_start(out=out_t[:], in_=score_sb[:, 0:1]).then_inc(
                    sem_out, 16
                )
                sync.wait_ge(sem_out, 16)

            @block.vector
            def _(vector):
                vcnt = 0

                def vop(inst, sem=None, val=1):
                    nonlocal vcnt
                    if sem is None:
                        inst.then_inc(sem_v, 1)
                        vcnt += 1
                    else:
                        inst.then_inc(sem, val)
                    return inst

                def vwait():
                    vector.wait_ge(sem_v, vcnt)

                vop(vector.memset(warm[:], 1.0), sem=sem_wm)
                vop(vector.memset(eps_sb[:], EPS))
                for n, t in enumerate(ACT_HI):
                    vop(vector.memset(bias_sb[:, n : n + 1], -(16.0 * t - 0.5)))
                # one-time ones planes (t=0 / s=0); never rewritten
                for n in range(2):
                    vop(vector.memset(plane(hi_b[n], 0), 1.0))
                    vop(vector.memset(plane(lo_b[n], 0), 1.0))
                for gh in range(reps * NHALF):
                    r, k = divmod(gh, NHALF)
                    b = k % 2
                    vector.wait_ge(sem_dma[b], 48 * (gh // 2 + 1))
                    if gh >= 2:
                        vector.wait_ge(sem_peh, gh - 1)  # plane bufs b free
                    if gh >= 2:
                        vector.wait_ge(sem_pla, gh - 1)  # ACT done with t_y[b]
                    vwait()  # prior half's DVE work (WAR on t_a/u16)
                    R = rgb[b][:, 0:HALF]
                    G = rgb[b][:, HALF : 2 * HALF]
                    B = rgb[b][:, 2 * HALF : 3 * HALF]
                    # y*255 = YSCL*((R*CR + G) + B*CB)
                    vop(
                        vector.scalar_tensor_tensor(
                            t_a[:], R, CR, G, Alu.mult, Alu.add
                        )
                    )
                    vwait()
                    vop(
                        vector.scalar_tensor_tensor(
                            t_y[b][:], B, CB, t_a[:], Alu.mult, Alu.add
                        ),
                        sem=sem_rgbf,
                    )
                    vector.wait_ge(sem_rgbf, gh + 1)
                    vop(
                        vector.tensor_scalar(
                            t_a[:], t_y[b][:], YSCL, MAGIC, Alu.mult, Alu.add
                        )
                    )
                    vwait()
                    vop(
                        vector.tensor_scalar(
                            u16[:], t_a[:], MAGIC, None, Alu.subtract
                        )
                    )
                    vwait()  # u16 ready
                    vop(vector.tensor_scalar(vlo[:], u16[:], 15, None, Alu.bitwise_and))
                    vwait()  # vlo ready
                    n_pl = len(DVE_HI) + 15
                    n_done = 0
                    for t in DVE_HI:
                        n_done += 1
                        inst = vector.tensor_scalar(
                            plane(hi_b[b], t), u16[:], 16 * t, None, Alu.is_ge
                        )
                        vop(inst, sem=sem_pl if n_done == n_pl else None, val=1)
                    for s in range(1, 16):
                        n_done += 1
                        inst = vector.tensor_scalar(
                            plane(lo_b[b], s), vlo[:], s, None, Alu.is_ge
                        )
                        vop(inst, sem=sem_pl if n_done == n_pl else None, val=1)

                    # ---- tail (after last half of this rep) ----
                    if k != NHALF - 1:
                        continue
                    vector.wait_ge(sem_peh, (r + 1) * NHALF)  # all hist MMs done
                    vector.wait_ge(sem_cdma, 48)  # mask loaded
                    for i in range(N_IMG):
                        gi = r * N_IMG + i
                        if gi >= 2:
                            vector.wait_ge(sem_smm, gi - 1)  # p_sb[i%2] free
                        vop(
                            vector.tensor_tensor(
                                p_sb[i % 2][:], psum_h[i][:], mask_sb[:], Alu.mult
                            ),
                            sem=sem_psb,
                        )
                    if r >= 1:
                        vector.wait_ge(sem_ln, r)  # prior rep's ACT read of hist4
                    for i in range(N_IMG):
                        gi = r * N_IMG + i
                        vector.wait_ge(sem_smm, gi + 1)
                        src = psum_o[i % 2][:].rearrange("j (l c) -> j l c", c=8)
                        vwait()
                        vector.wait_ge(sem_red, gi)  # mm4 free (prior copy done)
                        vop(vector.tensor_reduce(mm4[:], src, Axis.X, Alu.add))
                        vwait()
                        # lo recovery: column difference of the step family
                        vop(
                            vector.tensor_tensor(
                                hist4[:, 16 * i : 16 * i + 15],
                                mm4[:, 0:15],
                                mm4[:, 1:16],
                                Alu.subtract,
                            )
                        )
                        vop(
                            vector.tensor_copy(
                                hist4[:, 16 * i + 15 : 16 * i + 16], mm4[:, 15:16]
                            ),
                            sem=sem_red,
                        )
                    vector.wait_ge(sem_red, (r + 1) * N_IMG)
                    vector.wait_ge(sem_ln, r + 1)
                    vwait()
                    vop(vector.tensor_tensor(e4[:], hist4[:], ln4[:], Alu.mult))
                    vwait()
                    vop(
                        vector.tensor_reduce(
                            part[:],
                            e4[:].rearrange("p (i l) -> p i l", i=N_IMG),
                            Axis.X,
                            Alu.add,
                        ),
                        sem=sem_part,
                    )
                    vector.wait_ge(sem_sm, r + 1)
                    vop(
                        vector.tensor_scalar(
                            score_sb[:],
                            psum_s[:],
                            -1.0 / (NPIX * LN2),
                            None,
                            Alu.mult,
                        ),
                        sem=sem_sc,
                    )

            @block.tensor
            def _(tensor):
                for r in range(reps):
                    for k in range(NHALF):
                        i, h = divmod(k, 2)
                        b = k % 2
                        gh = r * NHALF + k
                        tensor.wait_ge(sem_pl, gh + 1)
                        tensor.wait_ge(sem_pla, gh + 1)
                        if h == 0 and r >= 1:
                            # psum_h[i] free only after prior rep's mask-mult
                            tensor.wait_ge(sem_psb, (r - 1) * N_IMG + i + 1)
                        last = None
                        for g in range(NGRP):
                            last = tensor.matmul(
                                psum_h[i][:],
                                lhsT=hi_b[b][:, 128 * g : 128 * (g + 1)],
                                rhs=lo_b[b][:, 128 * g : 128 * (g + 1)],
                                start=(h == 0 and g == 0),
                                stop=(h == 1 and g == NGRP - 1),
                            )
                        last.then_inc(sem_peh, 1)

                    tensor.wait_ge(sem_cdma, 48)
                    for i in range(N_IMG):
                        gi = r * N_IMG + i
                        tensor.wait_ge(sem_psb, gi + 1)
                        if gi >= 2:
                            tensor.wait_ge(sem_red, gi - 1)  # psum_o[i%2] free
                        tensor.matmul(
                            psum_o[i % 2][:],
                            lhsT=sel_sb[:],
                            rhs=p_sb[i % 2][:],
                            start=True,
                            stop=True,
                        ).then_inc(sem_smm, 1)
                    tensor.wait_ge(sem_part, r + 1)
                    if r >= 1:
                        tensor.wait_ge(sem_sc, r)  # psum_s free after DVE read
                    tensor.matmul(
                        psum_s[:],
                        lhsT=part[:],
                        rhs=ones_sb[:],
                        start=True,
                        stop=True,
                    ).then_inc(sem_sm, 1)

            @block.scalar
            def _(scalar):
                # warm up the Ln/Sign tables early
                scalar.wait_ge(sem_wm, 1)
                scalar.activation(warm[:], warm[:], Act.Ln, bias=1.0, scale=0.0)
                for gh in range(reps * NHALF):
                    r, k = divmod(gh, NHALF)
                    b = k % 2
                    scalar.wait_ge(sem_rgbf, gh + 1)  # m3 (t_a) ready
                    if gh >= 2:
                        scalar.wait_ge(sem_peh, gh - 1)  # plane bufs b free
                    for n, t in enumerate(ACT_HI):
                        inst = scalar.activation(
                            plane(hi_b[b], t),
                            t_y[b][:],
                            Act.Sign,
                            bias=bias_sb[:, n : n + 1],
                            scale=YSCL,
                        )
                        if n == len(ACT_HI) - 1:
                            inst.then_inc(sem_pla, 1)
                    # ---- per-rep Ln ----
                    if k == NHALF - 1:
                        scalar.wait_ge(sem_red, (r + 1) * N_IMG)
                        scalar.activation(
                            ln4[:],
                            hist4[:],
                            Act.Ln,
                            bias=eps_sb[:],
                            scale=1.0 / NPIX,
                        ).then_inc(sem_ln, 1)

    return nc


_NC_CACHE = {}


def _get_nc(reps=1):
    if reps not in _NC_CACHE:
        _NC_CACHE[reps] = build_bass(reps)
    return _NC_CACHE[reps]


def consts():
    # psum row index m = t*8 + c (t = hi plane, c = col-in-group).
    # F[t, a] = f_t(a) over hi-nibble values a; sel bakes W = F^-1 so the
    # selector matmul yields true per-hi-value counts from the mixed family.
    F = np.zeros((16, 16), np.float64)
    F[0, :] = 1.0
    for t in range(1, 16):
        step = (np.arange(16) >= t).astype(np.float64)
        F[t, :] = 2.0 * step - 1.0 if t in ACT_HI else step
    Wr = np.linalg.inv(F)  # [j', t]
    assert np.abs(Wr @ F - np.eye(16)).max() < 1e-9
    sel = np.zeros((P, 16), np.float32)
    for k in range(P):
        sel[k, :] = Wr[:, k // 8]
    mask = np.zeros((P, P), np.float32)
    for k in range(P):
        mask[k, k % 8 :: 8] = 1.0
    ones16 = np.ones((16, 1), np.float32)
    return sel, mask, ones16


def kernel(x):
    x = np.ascontiguousarray(np.asarray(x, dtype=np.float32))
    assert x.shape == (N_IMG * N_CORES, 3, H, W)
    from concourse.bass_utils import run_bass_kernel_spmd

    nc = _get_nc()
    sel, mask, ones16 = consts()
    in_maps = [
        {
            "x": np.ascontiguousarray(x[N_IMG * i : N_IMG * (i + 1)]),
            "sel": sel,
            "mask": mask,
            "ones16": ones16,
        }
        for i in range(N_CORES)
    ]
    res = run_bass_kernel_spmd(nc, in_maps, core_ids=list(range(N_CORES)))
    return np.concatenate([res.results[i]["out"] for i in range(N_CORES)])


# revision 20
# speedup vs baseline: 25.9141x; 1.0305x over previous
"""Per-image 256-bin luma-histogram entropy on Trainium2 (Bass, 8-core SPMD).

Input  x: (32, 3, 512, 512) fp32 RGB in [0,1]
Output   : (32,) fp32 entropy scores

Sharding: pure data parallel — batch split 4 images per NeuronCore, no
cross-core communication.

Per-core algorithm (4 images, processed as 8 half-images of [128,1024]):
  y = (0.299 R + 0.587 G + 0.114 B), m = y*255 RNE-rounded -> u in [0,255]
  (int16, RNE via the +-1.5*2^23 magic-add trick).

  Histogram via step-function factor planes (bf16, blocked layout
  free index = g*128 + t*8 + c, g = 8-col group, t = plane, c = col):
    hi side (planes t=0..15):  f_0 = 1 (one-time memset);
       t in DVE_HI:  f_t = (u >= 16t)          [DVE is_ge, {0,1}]
       t in ACT_HI:  f_t = sign(255*y-(16t-.5)) [ScalarE Sign, {-1,+1}]
    lo side (planes s=0..15):  g_0 = 1 (memset);
       g_s = ((u & 15) >= s)                   [DVE fused and+is_ge]
  TensorE contracts 8-col groups: lhsT/rhs = contiguous 128-col slices of
  the hi/lo plane buffers, accumulating PSUM[128,128]; entries with c==c'
  hold M_c[t,s] partial sums, c!=c' blocks are garbage.
  Fold: DVE multiply by block-diag mask (c==c'), then selector matmul
  whose constant bakes in W = F^-1 (recovery of hi one-hot counts from the
  mixed step/sign family), then grouped free-dim reduce over c' -> M'[j,s].
  Lo recovery is a column difference: J[:,s] = M'[:,s] - M'[:,s+1],
  J[:,15] = M'[:,15]  (G is the step family).
  entropy: ACT Ln(J/N + eps); e = J * ln; reduce; ones-matmul partition
  fold; score = -sum(e) / (N*ln2).

Engine sync: same-engine RAW/WAR needs explicit sem edges (engine
write-completion is async w.r.t. next-instruction issue; cross-engine
consumers inherit per-engine completion order). Each DVE op incs exactly
one sem: sem_v by default, or its cross-engine signal sem.
"""

from contextlib import ExitStack

import numpy as np

N_IMG = 4  # images per core
N_CORES = 8
H = 512
W = 512
P = 128  # SBUF partitions
HALF = 1024  # pixel columns per half-image ([128, 1024] = 131072 px)
NPIX = H * W  # pixels per image
EPS = 1e-8
LN2 = 0.6931471805599453
MAGIC = 12582912.0  # 1.5 * 2**23: add+sub performs fp32 round-to-nearest-even
CR = float(np.float32(0.299) / np.float32(0.587))
CB = float(np.float32(0.114) / np.float32(0.587))
YSCL = float(np.float32(0.587) * np.float32(255.0))
NHALF = N_IMG * 2
NGRP = HALF // 8  # 8-column matmul groups per half (128 cols each op)

# hi-plane split between engines (t=1..15; t=0 is the memset ones plane)
ACT_HI = tuple(range(7, 16))  # planes computed on ScalarE as sign (+-1)
DVE_HI = tuple(t for t in range(1, 16) if t not in ACT_HI)


def build_bass(reps=1):
    """Build the per-core Bass program. reps>1 repeats the whole pipeline
    (for marginal-cost HW timing); semaphore thresholds are offset per rep."""
    import concourse.bass as bass
    import concourse.mybir as mybir

    f32 = mybir.dt.float32
    bf16 = mybir.dt.bfloat16
    i16 = mybir.dt.int16
    Alu = mybir.AluOpType
    Act = mybir.ActivationFunctionType
    Axis = mybir.AxisListType

    nc = bass.Bass()

    x_t = nc.dram_tensor("x", [N_IMG, 3, H, W], f32, kind="ExternalInput")
    sel_t = nc.dram_tensor("sel", [P, 16], f32, kind="ExternalInput")
    mask_t = nc.dram_tensor("mask", [P, P], f32, kind="ExternalInput")
    ones_t = nc.dram_tensor("ones16", [16, 1], f32, kind="ExternalInput")
    out_t = nc.dram_tensor("out", [N_IMG], f32, kind="ExternalOutput")

    ctx = ExitStack()
    with ctx:
        # SBUF
        rgb = [
            ctx.enter_context(nc.sbuf_tensor(f"rgb{n}", [P, 3 * HALF], f32))
            for n in range(2)
        ]
        t_a = ctx.enter_context(nc.sbuf_tensor("t_a", [P, HALF], f32))
        t_y = [
            ctx.enter_context(nc.sbuf_tensor(f"t_y{n}", [P, HALF], f32))
            for n in range(2)
        ]
        u16 = ctx.enter_context(nc.sbuf_tensor("u16", [P, HALF], i16))
        vlo = ctx.enter_context(nc.sbuf_tensor("vlo", [P, HALF], i16))
        hi_b = [
            ctx.enter_context(nc.sbuf_tensor(f"hi{n}", [P, 16 * HALF], bf16))
            for n in range(2)
        ]
        lo_b = [
            ctx.enter_context(nc.sbuf_tensor(f"lo{n}", [P, 16 * HALF], bf16))
            for n in range(2)
        ]
        sel_sb = ctx.enter_context(nc.sbuf_tensor("sel_sb", [P, 16], f32))
        mask_sb = ctx.enter_context(nc.sbuf_tensor("mask_sb", [P, P], f32))
        ones_sb = ctx.enter_context(nc.sbuf_tensor("ones_sb", [16, 1], f32))
        p_sb = [
            ctx.enter_context(nc.sbuf_tensor(f"p_sb{n}", [P, P], f32))
            for n in range(2)
        ]
        mm4 = ctx.enter_context(nc.sbuf_tensor("mm4", [16, 16], f32))
        hist4 = ctx.enter_context(nc.sbuf_tensor("hist4", [16, 16 * N_IMG], f32))
        ln4 = ctx.enter_context(nc.sbuf_tensor("ln4", [16, 16 * N_IMG], f32))
        e4 = ctx.enter_context(nc.sbuf_tensor("e4", [16, 16 * N_IMG], f32))
        part = ctx.enter_context(nc.sbuf_tensor("part", [16, N_IMG], f32))
        score_sb = ctx.enter_context(nc.sbuf_tensor("score_sb", [N_IMG, 1], f32))
        warm = ctx.enter_context(nc.sbuf_tensor("warm", [1, 2], f32))
        eps_sb = ctx.enter_context(nc.sbuf_tensor("eps_sb", [16, 1], f32))
        bias_sb = ctx.enter_context(
            nc.sbuf_tensor("bias_sb", [P, len(ACT_HI)], f32)
        )

        # PSUM
        psum_h = [
            ctx.enter_context(nc.psum_tensor(f"psum_h{n}", [P, P], f32))
            for n in range(N_IMG)
        ]
        psum_o = [
            ctx.enter_context(nc.psum_tensor(f"psum_o{n}", [16, P], f32))
            for n in range(2)
        ]
        psum_s = ctx.enter_context(nc.psum_tensor("psum_s", [N_IMG, 1], f32))

        # semaphores
        sem_dma = [
            ctx.enter_context(nc.semaphore(f"dma_in{n}")) for n in range(2)
        ]
        sem_cdma = ctx.enter_context(nc.semaphore("const_dma"))
        sem_rgbf = ctx.enter_context(nc.semaphore("rgb_free"))
        sem_pl = ctx.enter_context(nc.semaphore("planes"))
        sem_pla = ctx.enter_context(nc.semaphore("planes_act"))
        sem_peh = ctx.enter_context(nc.semaphore("pe_half"))
        sem_psb = ctx.enter_context(nc.semaphore("psb"))
        sem_smm = ctx.enter_context(nc.semaphore("selmm"))
        sem_red = ctx.enter_context(nc.semaphore("red"))
        sem_ln = ctx.enter_context(nc.semaphore("ln"))
        sem_part = ctx.enter_context(nc.semaphore("part"))
        sem_sm = ctx.enter_context(nc.semaphore("scoremm"))
        sem_sc = ctx.enter_context(nc.semaphore("score"))
        sem_out = ctx.enter_context(nc.semaphore("out_dma"))
        sem_v = ctx.enter_context(nc.semaphore("dve_chain"))
        sem_wm = ctx.enter_context(nc.semaphore("warm"))

        def x_half_ap(i, c, h):
            # [512,512] -> [128, 2048] (4 consecutive rows per partition), half h
            a = x_t[i, c].rearrange("(p r) w -> p (r w)", r=4)
            return a[:, h * HALF : (h + 1) * HALF]

        def plane(buf, t):
            # blocked plane slot t of a hi/lo buffer: [128, NGRP, 8] strided
            return buf[:].rearrange("p (g j c) -> p g j c", j=16, c=8)[:, :, t, :]

        with nc.Block() as block:

            @block.sync
            def _(sync):
                sync.dma_start(out=sel_sb[:], in_=sel_t[:]).then_inc(sem_cdma, 16)
                sync.dma_start(out=mask_sb[:], in_=mask_t[:]).then_inc(sem_cdma, 16)
                sync.dma_start(out=ones_sb[:], in_=ones_t[:]).then_inc(sem_cdma, 16)
                for r in range(reps):
                    for k in range(NHALF):
                        i, h = divmod(k, 2)
                        b = k % 2
                        gh = r * NHALF + k
                        if gh >= 2:
                            sync.wait_ge(sem_rgbf, gh - 1)
                        for c in range(3):
                            sync.dma_start(
                                out=rgb[b][:, c * HALF : (c + 1) * HALF],
                                in_=x_half_ap(i, c, h),
                            ).then_inc(sem_dma[b], 16)
                sync.wait_ge(sem_sc, reps)
                sync.dma_start(out=out_t[:], in_=score_sb[:, 0:1]).then_inc(
                    sem_out, 16
                )
                sync.wait_ge(sem_out, 16)

            @block.vector
            def _(vector):
                vcnt = 0

                def vop(inst, sem=None, val=1):
                    nonlocal vcnt
                    if sem is None:
                        inst.then_inc(sem_v, 1)
                        vcnt += 1
                    else:
                        inst.then_inc(sem, val)
                    return inst

                def vwait():
                    vector.wait_ge(sem_v, vcnt)

                vop(vector.memset(warm[:], 1.0), sem=sem_wm)
                vop(vector.memset(eps_sb[:], EPS))
                for n, t in enumerate(ACT_HI):
                    vop(vector.memset(bias_sb[:, n : n + 1], -(16.0 * t - 0.5)))
                # one-time ones planes (t=0 / s=0); never rewritten
                for n in range(2):
                    vop(vector.memset(plane(hi_b[n], 0), 1.0))
                    vop(vector.memset(plane(lo_b[n], 0), 1.0))
                for gh in range(reps * NHALF):
                    r, k = divmod(gh, NHALF)
                    b = k % 2
                    vector.wait_ge(sem_dma[b], 48 * (gh // 2 + 1))
                    if gh >= 2:
                        vector.wait_ge(sem_peh, gh - 1)  # plane bufs b free
                    if gh >= 2:
                        vector.wait_ge(sem_pla, gh - 1)  # ACT done with t_y[b]
                    vwait()  # prior half's DVE work (WAR on t_a/u16)
                    R = rgb[b][:, 0:HALF]
                    G = rgb[b][:, HALF : 2 * HALF]
                    B = rgb[b][:, 2 * HALF : 3 * HALF]
                    # y*255 = YSCL*((R*CR + G) + B*CB)
                    vop(
                        vector.scalar_tensor_tensor(
                            t_a[:], R, CR, G, Alu.mult, Alu.add
                        )
                    )
                    vwait()
                    vop(
                        vector.scalar_tensor_tensor(
                            t_y[b][:], B, CB, t_a[:], Alu.mult, Alu.add
                        ),
                        sem=sem_rgbf,
                    )
                    vector.wait_ge(sem_rgbf, gh + 1)
                    vop(
                        vector.tensor_scalar(
                            t_a[:], t_y[b][:], YSCL, MAGIC, Alu.mult, Alu.add
                        )
                    )
                    vwait()
                    vop(
                        vector.tensor_scalar(
                            u16[:], t_a[:], MAGIC, None, Alu.subtract
                        )
                    )
                    vwait()  # u16 ready
                    vop(vector.tensor_scalar(vlo[:], u16[:], 15, None, Alu.bitwise_and))
                    vwait()  # vlo ready
                    n_pl = len(DVE_HI) + 15
                    n_done = 0
                    for t in DVE_HI:
                        n_done += 1
                        inst = vector.tensor_scalar(
                            plane(hi_b[b], t), u16[:], 16 * t, None, Alu.is_ge
                        )
                        vop(inst, sem=sem_pl if n_done == n_pl else None, val=1)
                    for s in range(1, 16):
                        n_done += 1
                        inst = vector.tensor_scalar(
                            plane(lo_b[b], s), vlo[:], s, None, Alu.is_ge
                        )
                        vop(inst, sem=sem_pl if n_done == n_pl else None, val=1)

                    # ---- incremental per-image tail, interleaved ----
                    # TA(i): mask-mult psum_h[i] -> p_sb   (after half 2i+2)
                    # TB(i): reduce+col-diff -> hist4      (after half 2i+3)
                    def TA(i):
                        gi = r * N_IMG + i
                        vector.wait_ge(sem_peh, r * NHALF + 2 * (i + 1))
                        if gi >= 2:
                            vector.wait_ge(sem_smm, gi - 1)  # p_sb[i%2] free
                        vop(
                            vector.tensor_tensor(
                                p_sb[i % 2][:], psum_h[i][:], mask_sb[:], Alu.mult
                            ),
                            sem=sem_psb,
                        )

                    def TB(i):
                        gi = r * N_IMG + i
                        if i == 0 and r >= 1:
                            vector.wait_ge(sem_ln, r)  # prior rep ACT read hist4
                        vector.wait_ge(sem_smm, gi + 1)
                        src = psum_o[i % 2][:].rearrange("j (l c) -> j l c", c=8)
                        vwait()
                        vector.wait_ge(sem_red, gi)  # mm4 free (prior copy done)
                        vop(vector.tensor_reduce(mm4[:], src, Axis.X, Alu.add))
                        vwait()
                        vop(
                            vector.tensor_tensor(
                                hist4[:, 16 * i : 16 * i + 15],
                                mm4[:, 0:15],
                                mm4[:, 1:16],
                                Alu.subtract,
                            )
                        )
                        vop(
                            vector.tensor_copy(
                                hist4[:, 16 * i + 15 : 16 * i + 16], mm4[:, 15:16]
                            ),
                            sem=sem_red,
                        )

                    if k >= 2 and k % 2 == 0:
                        if gh == 2:
                            vector.wait_ge(sem_cdma, 48)  # mask loaded
                        TA(k // 2 - 1)
                    if k >= 3 and k % 2 == 1:
                        TB(k // 2 - 1)
                    if k != NHALF - 1:
                        continue
                    TA(N_IMG - 1)
                    TB(N_IMG - 1)
                    # ---- entropy stage ----
                    vector.wait_ge(sem_ln, r + 1)
                    vwait()
                    vop(vector.tensor_tensor(e4[:], hist4[:], ln4[:], Alu.mult))
                    vwait()
                    vop(
                        vector.tensor_reduce(
                            part[:],
                            e4[:].rearrange("p (i l) -> p i l", i=N_IMG),
                            Axis.X,
                            Alu.add,
                        ),
                        sem=sem_part,
                    )
                    vector.wait_ge(sem_sm, r + 1)
                    vop(
                        vector.tensor_scalar(
                            score_sb[:],
                            psum_s[:],
                            -1.0 / (NPIX * LN2),
                            None,
                            Alu.mult,
                        ),
                        sem=sem_sc,
                    )

            @block.tensor
            def _(tensor):
                for r in range(reps):

                    def selmm(i):
                        gi = r * N_IMG + i
                        tensor.wait_ge(sem_psb, gi + 1)
                        if gi >= 2:
                            tensor.wait_ge(sem_red, gi - 1)  # psum_o[i%2] free
                        tensor.matmul(
                            psum_o[i % 2][:],
                            lhsT=sel_sb[:],
                            rhs=p_sb[i % 2][:],
                            start=True,
                            stop=True,
                        ).then_inc(sem_smm, 1)

                    for k in range(NHALF):
                        i, h = divmod(k, 2)
                        b = k % 2
                        gh = r * NHALF + k
                        tensor.wait_ge(sem_pl, gh + 1)
                        tensor.wait_ge(sem_pla, gh + 1)
                        if h == 0 and r >= 1:
                            # psum_h[i] free only after prior rep's mask-mult
                            tensor.wait_ge(sem_psb, (r - 1) * N_IMG + i + 1)
                        last = None
                        for g in range(NGRP):
                            last = tensor.matmul(
                                psum_h[i][:],
                                lhsT=hi_b[b][:, 128 * g : 128 * (g + 1)],
                                rhs=lo_b[b][:, 128 * g : 128 * (g + 1)],
                                start=(h == 0 and g == 0),
                                stop=(h == 1 and g == NGRP - 1),
                            )
                        last.then_inc(sem_peh, 1)
                        if k >= 2 and k % 2 == 0:
                            tensor.wait_ge(sem_cdma, 48)
                            selmm(k // 2 - 1)

                    selmm(N_IMG - 1)
                    tensor.wait_ge(sem_part, r + 1)
                    if r >= 1:
                        tensor.wait_ge(sem_sc, r)  # psum_s free after DVE read
                    tensor.matmul(
                        psum_s[:],
                        lhsT=part[:],
                        rhs=ones_sb[:],
                        start=True,
                        stop=True,
                    ).then_inc(sem_sm, 1)

            @block.scalar
            def _(scalar):
                # warm up the Ln/Sign tables early
                scalar.wait_ge(sem_wm, 1)
                scalar.activation(warm[:], warm[:], Act.Ln, bias=1.0, scale=0.0)
                for gh in range(reps * NHALF):
                    r, k = divmod(gh, NHALF)
                    b = k % 2
                    scalar.wait_ge(sem_rgbf, gh + 1)  # m3 (t_a) ready
                    if gh >= 2:
                        scalar.wait_ge(sem_peh, gh - 1)  # plane bufs b free
                    for n, t in enumerate(ACT_HI):
                        inst = scalar.activation(
                            plane(hi_b[b], t),
                            t_y[b][:],
                            Act.Sign,
                            bias=bias_sb[:, n : n + 1],
                            scale=YSCL,
                        )
                        if n == len(ACT_HI) - 1:
                            inst.then_inc(sem_pla, 1)
                    # ---- per-rep Ln ----
                    if k == NHALF - 1:
                        scalar.wait_ge(sem_red, (r + 1) * N_IMG)
                        scalar.activation(
                            ln4[:],
                            hist4[:],
                            Act.Ln,
                            bias=eps_sb[:],
                            scale=1.0 / NPIX,
                        ).then_inc(sem_ln, 1)

    return nc


_NC_CACHE = {}


def _get_nc(reps=1):
    if reps not in _NC_CACHE:
        _NC_CACHE[reps] = build_bass(reps)
    return _NC_CACHE[reps]


def consts():
    # psum row index m = t*8 + c (t = hi plane, c = col-in-group).
    # F[t, a] = f_t(a) over hi-nibble values a; sel bakes W = F^-1 so the
    # selector matmul yields true per-hi-value counts from the mixed family.
    F = np.zeros((16, 16), np.float64)
    F[0, :] = 1.0
    for t in range(1, 16):
        step = (np.arange(16) >= t).astype(np.float64)
        F[t, :] = 2.0 * step - 1.0 if t in ACT_HI else step
    Wr = np.linalg.inv(F)  # [j', t]
    assert np.abs(Wr @ F - np.eye(16)).max() < 1e-9
    sel = np.zeros((P, 16), np.float32)
    for k in range(P):
        sel[k, :] = Wr[:, k // 8]
    mask = np.zeros((P, P), np.float32)
    for k in range(P):
        mask[k, k % 8 :: 8] = 1.0
    ones16 = np.ones((16, 1), np.float32)
    return sel, mask, ones16


def kernel(x):
    x = np.ascontiguousarray(np.asarray(x, dtype=np.float32))
    assert x.shape == (N_IMG * N_CORES, 3, H, W)
    from concourse.bass_utils import run_bass_kernel_spmd

    nc = _get_nc()
    sel, mask, ones16 = consts()
    in_maps = [
        {
            "x": np.ascontiguousarray(x[N_IMG * i : N_IMG * (i + 1)]),
            "sel": sel,
            "mask": mask,
            "ones16": ones16,
        }
        for i in range(N_CORES)
    ]
    res = run_bass_kernel_spmd(nc, in_maps, core_ids=list(range(N_CORES)))
    return np.concatenate([res.results[i]["out"] for i in range(N_CORES)])


# revision 21
# speedup vs baseline: 27.1722x; 1.0485x over previous
"""Per-image 256-bin luma-histogram entropy on Trainium2 (Bass, 8-core SPMD).

Input  x: (32, 3, 512, 512) fp32 RGB in [0,1]
Output   : (32,) fp32 entropy scores

Sharding: pure data parallel — batch split 4 images per NeuronCore, no
cross-core communication.

Per-core algorithm (4 images, processed as 8 half-images of [128,1024]):
  y = (0.299 R + 0.587 G + 0.114 B), m = y*255 RNE-rounded -> u in [0,255]
  (int16, RNE via the +-1.5*2^23 magic-add trick).

  Histogram via step-function factor planes (bf16, blocked layout
  free index = g*128 + t*8 + c, g = 8-col group, t = plane, c = col):
    hi side (planes t=0..15):  f_0 = 1 (one-time memset);
       t in DVE_HI:  f_t = (u >= 16t)          [DVE is_ge, {0,1}]
       t in ACT_HI:  f_t = sign(255*y-(16t-.5)) [ScalarE Sign, {-1,+1}]
    lo side (planes s=0..15):  g_0 = 1 (memset);
       g_s = ((u & 15) >= s)                   [DVE fused and+is_ge]
  TensorE contracts 8-col groups: lhsT/rhs = contiguous 128-col slices of
  the hi/lo plane buffers, accumulating PSUM[128,128]; entries with c==c'
  hold M_c[t,s] partial sums, c!=c' blocks are garbage.
  Fold: DVE multiply by block-diag mask (c==c'), then selector matmul
  whose constant bakes in W = F^-1 (recovery of hi one-hot counts from the
  mixed step/sign family), then grouped free-dim reduce over c' -> M'[j,s].
  Lo recovery is a column difference: J[:,s] = M'[:,s] - M'[:,s+1],
  J[:,15] = M'[:,15]  (G is the step family).
  entropy: ACT Ln(J/N + eps); e = J * ln; reduce; ones-matmul partition
  fold; score = -sum(e) / (N*ln2).

Engine sync: same-engine RAW/WAR needs explicit sem edges (engine
write-completion is async w.r.t. next-instruction issue; cross-engine
consumers inherit per-engine completion order). Each DVE op incs exactly
one sem: sem_v by default, or its cross-engine signal sem.
"""

from contextlib import ExitStack

import numpy as np

N_IMG = 4  # images per core
N_CORES = 8
H = 512
W = 512
P = 128  # SBUF partitions
HALF = 1024  # pixel columns per half-image ([128, 1024] = 131072 px)
NPIX = H * W  # pixels per image
EPS = 1e-8
LN2 = 0.6931471805599453
MAGIC = 12582912.0  # 1.5 * 2**23: add+sub performs fp32 round-to-nearest-even
CR = float(np.float32(0.299) / np.float32(0.587))
CB = float(np.float32(0.114) / np.float32(0.587))
YSCL = float(np.float32(0.587) * np.float32(255.0))
NHALF = N_IMG * 2
NGRP = HALF // 8  # 8-column matmul groups per half (128 cols each op)

# hi-plane split between engines (t=1..15; t=0 is the memset ones plane)
ACT_HI = tuple(range(7, 16))  # planes computed on ScalarE as sign (+-1)
DVE_HI = tuple(t for t in range(1, 16) if t not in ACT_HI)


def build_bass(reps=1):
    """Build the per-core Bass program. reps>1 repeats the whole pipeline
    (for marginal-cost HW timing); semaphore thresholds are offset per rep."""
    import concourse.bass as bass
    import concourse.mybir as mybir

    f32 = mybir.dt.float32
    bf16 = mybir.dt.bfloat16
    i16 = mybir.dt.int16
    Alu = mybir.AluOpType
    Act = mybir.ActivationFunctionType
    Axis = mybir.AxisListType

    nc = bass.Bass()

    x_t = nc.dram_tensor("x", [N_IMG, 3, H, W], f32, kind="ExternalInput")
    sel_t = nc.dram_tensor("sel", [P, 16], f32, kind="ExternalInput")
    mask_t = nc.dram_tensor("mask", [P, P], f32, kind="ExternalInput")
    ones_t = nc.dram_tensor("ones16", [16, 1], f32, kind="ExternalInput")
    out_t = nc.dram_tensor("out", [N_IMG], f32, kind="ExternalOutput")

    ctx = ExitStack()
    with ctx:
        # SBUF
        rgb = [
            ctx.enter_context(nc.sbuf_tensor(f"rgb{n}", [P, 3 * HALF], f32))
            for n in range(2)
        ]
        t_a = ctx.enter_context(nc.sbuf_tensor("t_a", [P, HALF], f32))
        t_y = [
            ctx.enter_context(nc.sbuf_tensor(f"t_y{n}", [P, HALF], f32))
            for n in range(2)
        ]
        u16 = ctx.enter_context(nc.sbuf_tensor("u16", [P, HALF], i16))
        vlo = ctx.enter_context(nc.sbuf_tensor("vlo", [P, HALF], i16))
        hi_b = [
            ctx.enter_context(nc.sbuf_tensor(f"hi{n}", [P, 16 * HALF], bf16))
            for n in range(2)
        ]
        lo_b = [
            ctx.enter_context(nc.sbuf_tensor(f"lo{n}", [P, 16 * HALF], bf16))
            for n in range(2)
        ]
        sel_sb = ctx.enter_context(nc.sbuf_tensor("sel_sb", [P, 16], f32))
        mask_sb = ctx.enter_context(nc.sbuf_tensor("mask_sb", [P, P], f32))
        ones_sb = ctx.enter_context(nc.sbuf_tensor("ones_sb", [16, 1], f32))
        p_sb = [
            ctx.enter_context(nc.sbuf_tensor(f"p_sb{n}", [P, P], f32))
            for n in range(2)
        ]
        mm4 = ctx.enter_context(nc.sbuf_tensor("mm4", [16, 16], f32))
        hist4 = ctx.enter_context(nc.sbuf_tensor("hist4", [16, 16 * N_IMG], f32))
        ln4 = ctx.enter_context(nc.sbuf_tensor("ln4", [16, 16 * N_IMG], f32))
        e4 = ctx.enter_context(nc.sbuf_tensor("e4", [16, 16 * N_IMG], f32))
        part = ctx.enter_context(nc.sbuf_tensor("part", [16, N_IMG], f32))
        score_sb = ctx.enter_context(nc.sbuf_tensor("score_sb", [N_IMG, 1], f32))
        warm = ctx.enter_context(nc.sbuf_tensor("warm", [1, 2], f32))
        eps_sb = ctx.enter_context(nc.sbuf_tensor("eps_sb", [16, 1], f32))
        bias_sb = ctx.enter_context(
            nc.sbuf_tensor("bias_sb", [P, len(ACT_HI)], f32)
        )

        # PSUM
        psum_h = [
            ctx.enter_context(nc.psum_tensor(f"psum_h{n}", [P, P], f32))
            for n in range(N_IMG)
        ]
        psum_o = [
            ctx.enter_context(nc.psum_tensor(f"psum_o{n}", [16, P], f32))
            for n in range(2)
        ]
        psum_s = ctx.enter_context(nc.psum_tensor("psum_s", [N_IMG, 1], f32))

        # semaphores
        sem_dma = [
            ctx.enter_context(nc.semaphore(f"dma_in{n}")) for n in range(2)
        ]
        sem_cdma = ctx.enter_context(nc.semaphore("const_dma"))
        sem_rgbf = ctx.enter_context(nc.semaphore("rgb_free"))
        sem_pl = ctx.enter_context(nc.semaphore("planes"))
        sem_pla = ctx.enter_context(nc.semaphore("planes_act"))
        sem_peh = ctx.enter_context(nc.semaphore("pe_half"))
        sem_psb = ctx.enter_context(nc.semaphore("psb"))
        sem_smm = ctx.enter_context(nc.semaphore("selmm"))
        sem_red = ctx.enter_context(nc.semaphore("red"))
        sem_ln = ctx.enter_context(nc.semaphore("ln"))
        sem_part = ctx.enter_context(nc.semaphore("part"))
        sem_sm = ctx.enter_context(nc.semaphore("scoremm"))
        sem_sc = ctx.enter_context(nc.semaphore("score"))
        sem_out = ctx.enter_context(nc.semaphore("out_dma"))
        sem_v = ctx.enter_context(nc.semaphore("dve_chain"))
        sem_wm = ctx.enter_context(nc.semaphore("warm"))

        def x_half_ap(i, c, h):
            # [512,512] -> [128, 2048] (4 consecutive rows per partition), half h
            a = x_t[i, c].rearrange("(p r) w -> p (r w)", r=4)
            return a[:, h * HALF : (h + 1) * HALF]

        import os

        probe_packed = os.environ.get("BASS_PROBE_PACKED") == "1"

        def plane(buf, t):
            if probe_packed:
                # timing probe: packed plane-major writes (WRONG results)
                return buf[:, t * HALF : (t + 1) * HALF]
            # blocked plane slot t of a hi/lo buffer: [128, NGRP, 8] strided
            return buf[:].rearrange("p (g j c) -> p g j c", j=16, c=8)[:, :, t, :]

        with nc.Block() as block:

            @block.sync
            def _(sync):
                sync.dma_start(out=sel_sb[:], in_=sel_t[:]).then_inc(sem_cdma, 16)
                sync.dma_start(out=mask_sb[:], in_=mask_t[:]).then_inc(sem_cdma, 16)
                sync.dma_start(out=ones_sb[:], in_=ones_t[:]).then_inc(sem_cdma, 16)
                for r in range(reps):
                    for k in range(NHALF):
                        i, h = divmod(k, 2)
                        b = k % 2
                        gh = r * NHALF + k
                        if gh >= 2:
                            sync.wait_ge(sem_rgbf, gh - 1)
                        for c in range(3):
                            sync.dma_start(
                                out=rgb[b][:, c * HALF : (c + 1) * HALF],
                                in_=x_half_ap(i, c, h),
                            ).then_inc(sem_dma[b], 16)
                sync.wait_ge(sem_sc, reps)
                sync.dma_start(out=out_t[:], in_=score_sb[:, 0:1]).then_inc(
                    sem_out, 16
                )
                sync.wait_ge(sem_out, 16)

            @block.vector
            def _(vector):
                vcnt = 0

                def vop(inst, sem=None, val=1):
                    nonlocal vcnt
                    if sem is None:
                        inst.then_inc(sem_v, 1)
                        vcnt += 1
                    else:
                        inst.then_inc(sem, val)
                    return inst

                def vwait():
                    vector.wait_ge(sem_v, vcnt)

                vop(vector.memset(warm[:], 1.0), sem=sem_wm)
                vop(vector.memset(eps_sb[:], EPS))
                for n, t in enumerate(ACT_HI):
                    vop(vector.memset(bias_sb[:, n : n + 1], -(16.0 * t - 0.5)))
                # one-time ones planes (t=0 / s=0); never rewritten
                for n in range(2):
                    vop(vector.memset(plane(hi_b[n], 0), 1.0))
                    vop(vector.memset(plane(lo_b[n], 0), 1.0))
                for gh in range(reps * NHALF):
                    r, k = divmod(gh, NHALF)
                    b = k % 2
                    vector.wait_ge(sem_dma[b], 48 * (gh // 2 + 1))
                    if gh >= 2:
                        vector.wait_ge(sem_peh, gh - 1)  # plane bufs b free
                    if gh >= 2:
                        vector.wait_ge(sem_pla, gh - 1)  # ACT done with t_y[b]
                    vwait()  # prior half's DVE work (WAR on t_a/u16)
                    R = rgb[b][:, 0:HALF]
                    G = rgb[b][:, HALF : 2 * HALF]
                    B = rgb[b][:, 2 * HALF : 3 * HALF]
                    # y*255 = YSCL*((R*CR + G) + B*CB)
                    vop(
                        vector.scalar_tensor_tensor(
                            t_a[:], R, CR, G, Alu.mult, Alu.add
                        )
                    )
                    vwait()
                    vop(
                        vector.scalar_tensor_tensor(
                            t_y[b][:], B, CB, t_a[:], Alu.mult, Alu.add
                        ),
                        sem=sem_rgbf,
                    )
                    vector.wait_ge(sem_rgbf, gh + 1)
                    vop(
                        vector.tensor_scalar(
                            t_a[:], t_y[b][:], YSCL, MAGIC, Alu.mult, Alu.add
                        )
                    )
                    vwait()
                    vop(
                        vector.tensor_scalar(
                            u16[:], t_a[:], MAGIC, None, Alu.subtract
                        )
                    )
                    vwait()  # u16 ready
                    vop(vector.tensor_scalar(vlo[:], u16[:], 15, None, Alu.bitwise_and))
                    vwait()  # vlo ready
                    n_pl = len(DVE_HI) + 15
                    n_done = 0
                    for t in DVE_HI:
                        n_done += 1
                        inst = vector.tensor_scalar(
                            plane(hi_b[b], t), u16[:], 16 * t, None, Alu.is_ge
                        )
                        vop(inst, sem=sem_pl if n_done == n_pl else None, val=1)
                    for s in range(1, 16):
                        n_done += 1
                        inst = vector.tensor_scalar(
                            plane(lo_b[b], s), vlo[:], s, None, Alu.is_ge
                        )
                        vop(inst, sem=sem_pl if n_done == n_pl else None, val=1)

                    # ---- incremental per-image tail, interleaved ----
                    # TA(i): mask-mult psum_h[i] -> p_sb   (after half 2i+2)
                    # TB(i): reduce+col-diff -> hist4      (after half 2i+3)
                    def TA(i):
                        gi = r * N_IMG + i
                        vector.wait_ge(sem_peh, r * NHALF + 2 * (i + 1))
                        if gi >= 2:
                            vector.wait_ge(sem_smm, gi - 1)  # p_sb[i%2] free
                        vop(
                            vector.tensor_tensor(
                                p_sb[i % 2][:], psum_h[i][:], mask_sb[:], Alu.mult
                            ),
                            sem=sem_psb,
                        )

                    def TB(i):
                        gi = r * N_IMG + i
                        if i == 0 and r >= 1:
                            vector.wait_ge(sem_ln, r)  # prior rep ACT read hist4
                        vector.wait_ge(sem_smm, gi + 1)
                        src = psum_o[i % 2][:].rearrange("j (l c) -> j l c", c=8)
                        vwait()
                        vector.wait_ge(sem_red, gi)  # mm4 free (prior copy done)
                        vop(vector.tensor_reduce(mm4[:], src, Axis.X, Alu.add))
                        vwait()
                        vop(
                            vector.tensor_tensor(
                                hist4[:, 16 * i : 16 * i + 15],
                                mm4[:, 0:15],
                                mm4[:, 1:16],
                                Alu.subtract,
                            )
                        )
                        vop(
                            vector.tensor_copy(
                                hist4[:, 16 * i + 15 : 16 * i + 16], mm4[:, 15:16]
                            ),
                            sem=sem_red,
                        )

                    if k >= 2 and k % 2 == 0:
                        if gh == 2:
                            vector.wait_ge(sem_cdma, 48)  # mask loaded
                        TA(k // 2 - 1)
                    if k >= 3 and k % 2 == 1:
                        TB(k // 2 - 1)
                    if k != NHALF - 1:
                        continue
                    TA(N_IMG - 1)
                    TB(N_IMG - 1)
                    # ---- entropy stage ----
                    vector.wait_ge(sem_ln, r + 1)
                    vwait()
                    vop(vector.tensor_tensor(e4[:], hist4[:], ln4[:], Alu.mult))
                    vwait()
                    vop(
                        vector.tensor_reduce(
                            part[:],
                            e4[:].rearrange("p (i l) -> p i l", i=N_IMG),
                            Axis.X,
                            Alu.add,
                        ),
                        sem=sem_part,
                    )
                    vector.wait_ge(sem_sm, r + 1)
                    vop(
                        vector.tensor_scalar(
                            score_sb[:],
                            psum_s[:],
                            -1.0 / (NPIX * LN2),
                            None,
                            Alu.mult,
                        ),
                        sem=sem_sc,
                    )

            @block.tensor
            def _(tensor):
                for r in range(reps):

                    def selmm(i):
                        gi = r * N_IMG + i
                        tensor.wait_ge(sem_psb, gi + 1)
                        if gi >= 2:
                            tensor.wait_ge(sem_red, gi - 1)  # psum_o[i%2] free
                        tensor.matmul(
                            psum_o[i % 2][:],
                            lhsT=sel_sb[:],
                            rhs=p_sb[i % 2][:],
                            start=True,
                            stop=True,
                        ).then_inc(sem_smm, 1)

                    for k in range(NHALF):
                        i, h = divmod(k, 2)
                        b = k % 2
                        gh = r * NHALF + k
                        tensor.wait_ge(sem_pl, gh + 1)
                        tensor.wait_ge(sem_pla, gh + 1)
                        if h == 0 and r >= 1:
                            # psum_h[i] free only after prior rep's mask-mult
                            tensor.wait_ge(sem_psb, (r - 1) * N_IMG + i + 1)
                        last = None
                        for g in range(NGRP):
                            last = tensor.matmul(
                                psum_h[i][:],
                                lhsT=hi_b[b][:, 128 * g : 128 * (g + 1)],
                                rhs=lo_b[b][:, 128 * g : 128 * (g + 1)],
                                start=(h == 0 and g == 0),
                                stop=(h == 1 and g == NGRP - 1),
                            )
                        last.then_inc(sem_peh, 1)
                        if k >= 2 and k % 2 == 0:
                            tensor.wait_ge(sem_cdma, 48)
                            selmm(k // 2 - 1)

                    selmm(N_IMG - 1)
                    tensor.wait_ge(sem_part, r + 1)
                    if r >= 1:
                        tensor.wait_ge(sem_sc, r)  # psum_s free after DVE read
                    tensor.matmul(
                        psum_s[:],
                        lhsT=part[:],
                        rhs=ones_sb[:],
                        start=True,
                        stop=True,
                    ).then_inc(sem_sm, 1)

            @block.scalar
            def _(scalar):
                # warm up the Ln/Sign tables early
                scalar.wait_ge(sem_wm, 1)
                scalar.activation(warm[:], warm[:], Act.Ln, bias=1.0, scale=0.0)
                for gh in range(reps * NHALF):
                    r, k = divmod(gh, NHALF)
                    b = k % 2
                    scalar.wait_ge(sem_rgbf, gh + 1)  # m3 (t_a) ready
                    if gh >= 2:
                        scalar.wait_ge(sem_peh, gh - 1)  # plane bufs b free
                    for n, t in enumerate(ACT_HI):
                        inst = scalar.activation(
                            plane(hi_b[b], t),
                            t_y[b][:],
                            Act.Sign,
                            bias=bias_sb[:, n : n + 1],
                            scale=YSCL,
                        )
                        if n == len(ACT_HI) - 1:
                            inst.then_inc(sem_pla, 1)
                    # ---- per-rep Ln ----
                    if k == NHALF - 1:
                        scalar.wait_ge(sem_red, (r + 1) * N_IMG)
                        scalar.activation(
                            ln4[:],
                            hist4[:],
                            Act.Ln,
                            bias=eps_sb[:],
                            scale=1.0 / NPIX,
                        ).then_inc(sem_ln, 1)

    return nc


_NC_CACHE = {}


def _get_nc(reps=1):
    if reps not in _NC_CACHE:
        _NC_CACHE[reps] = build_bass(reps)
    return _NC_CACHE[reps]


def consts():
    # psum row index m = t*8 + c (t = hi plane, c = col-in-group).
    # F[t, a] = f_t(a) over hi-nibble values a; sel bakes W = F^-1 so the
    # selector matmul yields true per-hi-value counts from the mixed family.
    F = np.zeros((16, 16), np.float64)
    F[0, :] = 1.0
    for t in range(1, 16):
        step = (np.arange(16) >= t).astype(np.float64)
        F[t, :] = 2.0 * step - 1.0 if t in ACT_HI else step
    Wr = np.linalg.inv(F)  # [j', t]
    assert np.abs(Wr @ F - np.eye(16)).max() < 1e-9
    sel = np.zeros((P, 16), np.float32)
    for k in range(P):
        sel[k, :] = Wr[:, k // 8]
    mask = np.zeros((P, P), np.float32)
    for k in range(P):
        mask[k, k % 8 :: 8] = 1.0
    ones16 = np.ones((16, 1), np.float32)
    return sel, mask, ones16


def kernel(x):
    x = np.ascontiguousarray(np.asarray(x, dtype=np.float32))
    assert x.shape == (N_IMG * N_CORES, 3, H, W)
    from concourse.bass_utils import run_bass_kernel_spmd

    nc = _get_nc()
    sel, mask, ones16 = consts()
    in_maps = [
        {
            "x": np.ascontiguousarray(x[N_IMG * i : N_IMG * (i + 1)]),
            "sel": sel,
            "mask": mask,
            "ones16": ones16,
        }
        for i in range(N_CORES)
    ]
    res = run_bass_kernel_spmd(nc, in_maps, core_ids=list(range(N_CORES)))
    return np.concatenate([res.results[i]["out"] for i in range(N_CORES)])
